# revision 43
# baseline (speedup 1.0000x reference)
"""CommNetMLP forward pass on Trainium2 (Bass/Tile), 8-core data-parallel.

Model (per reference):
    x_enc = tanh(x @ W_enc + b_enc)                      [B, N, H]
    h = x_enc
    for i in range(P):
        comm = einsum('ji,bjm->bim', comm_mask, h)       sender-sum
        c = comm @ C_W[i] + C_b[i]
        h = tanh(x_enc + h @ f_W[i] + f_b[i] + c)
    value  = h @ val_W + val_b                           [B, N, 1]
    action = log_softmax(h @ head_W + head_b, -1)        [B, N, A]

Device strategy (per core, B_loc = 8 batches = 800 tokens):
  comm_mask is (in practice) a*ones + d*eye, so
      comm[b,i] = a*S[b] + d*h[b,i],  S[b] = sum_j h[b,j]
  and each pass folds to
      h' = tanh(x_enc + h @ (f_W + d*C_W) + (a*S @ C_W)|bcast + (f_b + C_b))
  Activations live H-major [128, 800] so every matmul contracts over
  partitions.  Per pass, per 400-token chunk (4 batches):
    - PSUM accumulates h @ W1, the x_enc skip (identity matmul; folded into
      W1 host-side for pass 0 where h == x_enc), and the comm term as ONE
      matmul with a step-0 broadcast rhs: psum += W2'.T stationary x S[:, b]
      streamed 100x per batch.  Chunk c only needs chunk c's S columns, so
      stages pipeline per-chunk with no cross-chunk barrier.
    - ScalarE applies tanh(psum + bias); VectorE reduces the per-batch
      sender sums S for the next stage.
  The head runs token-major [100, 8, 6]; log_softmax reduces along the free
  dim, with ln() computed on the VectorE via exponent/mantissa bit
  extraction + a Horner polynomial (avoids a second ACT table load).
  Matmuls use float32r (single-pass, ~TF32 precision; 4x faster than fp32).
  ScalarE preloads its activation table and dummy matmuls warm the PE's HAM
  clock gate during the input DMAs.
"""

import numpy as np
import ml_dtypes

import concourse.bass as bass
from concourse import bacc
import concourse.mybir as mybir
import concourse.tile as tile
from concourse.bass_utils import run_bass_kernel_spmd

# Problem shapes (fixed for this problem instance).
B, N, H, DIN, A, NPASS = 64, 100, 128, 128, 5, 2
NCORES = 8
BLOC = B // NCORES            # batches per core
T = BLOC * N                  # tokens per core
NCHUNK = 2
CHUNK = T // NCHUNK           # matmul free-dim chunk (<= 512, batch aligned)
BPC = CHUNK // N              # batches per chunk

WCOLS = 6 * H + (A + 1)       # packed weights: wenc|I|w1_0+I|w2_0|w1_1|w2_1|hvw

F32 = mybir.dt.float32
BF16 = mybir.dt.bfloat16
I32 = mybir.dt.int32

# Matmul / activation storage dtype ("f32", "bf16", or "f32r").
MM_MODE = "f32r"

AF = mybir.ActivationFunctionType
ALU = mybir.AluOpType
AX = mybir.AxisListType

LN2 = float(np.log(2.0))

def _ln_poly_coeffs(deg=5):
    # Chebyshev fit of ln(x) on [1, 2], converted to power basis.
    cheb = np.polynomial.chebyshev.Chebyshev.fit(
        np.linspace(1.0, 2.0, 2048), np.log(np.linspace(1.0, 2.0, 2048)), deg
    )
    return [float(c) for c in cheb.convert(kind=np.polynomial.Polynomial).coef]

LN_COEF = _ln_poly_coeffs()   # c0..c7


def _mm_dt():
    return {"f32": F32, "bf16": BF16, "f32r": mybir.dt.float32r}[MM_MODE]


def _np_dt():
    return {"f32": np.float32, "bf16": ml_dtypes.bfloat16, "f32r": np.float32}[
        MM_MODE
    ]


def _build(has_hvb: bool) -> bass.Bass:
    DT = _mm_dt()
    nc = bacc.Bacc("TRN2", target_bir_lowering=False, num_swdge_queues=2)

    xT = nc.dram_tensor("xT", [H, T], DT, kind="ExternalInput")
    wpack = nc.dram_tensor("wpack", [H, WCOLS], DT, kind="ExternalInput")
    bpack = nc.dram_tensor("bpack", [H, 1 + NPASS], F32, kind="ExternalInput")
    if has_hvb:
        hvb = nc.dram_tensor("hvb", [1, A + 1], F32, kind="ExternalInput")

    outp = nc.dram_tensor("outp", [N, BLOC, A + 1], F32, kind="ExternalOutput")

    with tile.TileContext(nc) as tc:
        with (
            tc.tile_pool(name="persist", bufs=1) as persist,
            tc.tile_pool(name="small", bufs=2) as small,
            tc.tile_pool(name="psch", bufs=4, space="PSUM") as psch,
            tc.tile_pool(name="pssm", bufs=1, space="PSUM") as pssm,
        ):
            sb_x = persist.tile([H, T], DT, name="sb_x")
            sb_xenc = persist.tile([H, T], DT, name="sb_xenc")
            sb_h1 = persist.tile([H, T], DT, name="sb_h1")
            sb_h2 = persist.tile([H, T], DT, name="sb_h2")
            sb_w = persist.tile([H, WCOLS], DT, name="sb_w")
            sb_b = persist.tile([H, 1 + NPASS], F32, name="sb_b")
            # Per-stage per-batch sender sums S[k, b] (DT, per-chunk reduces).
            sb_S = [
                persist.tile([H, BLOC], DT, name=f"sb_S{s}") for s in range(3)
            ]

            # wpack column layout (ordered by when each section is needed):
            # [wenc | I | w1_0 | w2_0 | w1_1 | w2_1 | hvw]
            w_enc = sb_w[:, 0:H]
            w_id = sb_w[:, H : 2 * H]
            w1 = [sb_w[:, 2 * H : 3 * H], sb_w[:, 4 * H : 5 * H]]
            w2 = [sb_w[:, 3 * H : 4 * H], sb_w[:, 5 * H : 6 * H]]
            w_hv = sb_w[:, 6 * H : 6 * H + A + 1]
            b_enc = sb_b[:, 0:1]

            # Warmup: force the exp_and_others ACT table load to start at t=0.
            scratch = persist.tile([H, 1], F32, name="scratch")
            nc.vector.memset(scratch[:], 0.0)
            warm = persist.tile([H, 1], F32, name="warm")
            nc.scalar.activation(out=warm[:], in_=scratch[:], func=AF.Tanh)

            # PE warm-up: keep the systolic array busy from t~0 so the HAM
            # clock gate releases (1.2 -> 2.4 GHz) before the real matmuls.
            zeros32 = persist.tile([H, 512], F32, name="zeros32")
            nc.vector.memset(zeros32[:], 0.0)
            zeros = persist.tile([H, 512], DT, name="zeros")
            nc.vector.tensor_copy(zeros[:], zeros32[:])
            ps_wm = pssm.tile([1, 512], F32, name="ps_wm", tag="ps_wm")
            for _ in range(4):
                nc.tensor.matmul(
                    ps_wm[:], zeros[:, 0:1], zeros[:], start=True, stop=True
                )

            # Input DMAs spread across per-engine DGE queues (each HWDGE DMA
            # pays ~625ns fixed on its issuing engine's queue; SWDGE ~1us).
            nc.sync.dma_start(out=sb_x[:, 0:CHUNK], in_=xT[:, 0:CHUNK])
            nc.sync.dma_start(out=sb_x[:, CHUNK:T], in_=xT[:, CHUNK:T])
            nc.scalar.dma_start(out=sb_w[:, 0 : 2 * H], in_=wpack[:, 0 : 2 * H])
            nc.gpsimd.dma_start(out=sb_b[:], in_=bpack[:, :])
            nc.gpsimd.dma_start(
                out=sb_w[:, 2 * H : 4 * H], in_=wpack[:, 2 * H : 4 * H]
            )
            nc.gpsimd.dma_start(
                out=sb_w[:, 4 * H : WCOLS], in_=wpack[:, 4 * H : WCOLS]
            )

            def tanh_and_sum(h_out, S_out, psums, bias):
                """Per-chunk tanh (ScalarE) + per-chunk per-batch sender-sum
                reduce (VectorE) writing S columns for the next stage."""
                for c in range(NCHUNK):
                    sl = slice(c * CHUNK, (c + 1) * CHUNK)
                    nc.scalar.activation(
                        out=h_out[:, sl], in_=psums[c][:], func=AF.Tanh,
                        bias=bias,
                    )
                    with nc.allow_low_precision(reason="f32r sender-sum"):
                        nc.vector.tensor_reduce(
                            S_out[:, c * BPC : (c + 1) * BPC],
                            h_out[:, sl].rearrange("k (b n) -> k b n", b=BPC),
                            axis=AX.X,
                            op=ALU.add,
                        )

            # Encoder.
            enc_ps = []
            for c in range(NCHUNK):
                sl = slice(c * CHUNK, (c + 1) * CHUNK)
                ps = psch.tile([H, CHUNK], F32, name="ps_mm", tag="ps_mm")
                nc.tensor.matmul(ps[:], w_enc, sb_x[:, sl], start=True, stop=True)
                enc_ps.append(ps)
            tanh_and_sum(sb_xenc, sb_S[0], enc_ps, b_enc)

            # Communication passes.  The comm term lands in PSUM as a single
            # matmul per chunk with a step-0 broadcast rhs:
            #   psum[hout, (b, n)] += sum_k W2'[k, hout] * S[k, b]
            # and chunk c only needs chunk c's own S columns, so the pass-to-
            # pass joint is fully per-chunk pipelined.
            for p in range(NPASS):
                h_in = sb_xenc if p == 0 else sb_h1
                h_out = sb_h1 if p == 0 else sb_h2

                ph = []
                for c in range(NCHUNK):
                    sl = slice(c * CHUNK, (c + 1) * CHUNK)
                    ps = psch.tile([H, CHUNK], F32, name="ps_mm", tag="ps_mm")
                    if p == 0:
                        # h_in == x_enc, so the identity skip is folded into
                        # w1[0] host-side (W1_0 + I): one matmul instead of 2.
                        nc.tensor.matmul(
                            ps[:], w1[p], h_in[:, sl], start=True, stop=False
                        )
                    else:
                        nc.tensor.matmul(
                            ps[:], w_id, sb_xenc[:, sl], start=True, stop=False
                        )
                        nc.tensor.matmul(
                            ps[:], w1[p], h_in[:, sl], start=False, stop=False
                        )
                    S_bc = (
                        sb_S[p][:, c * BPC : (c + 1) * BPC]
                        .unsqueeze(2)
                        .to_broadcast((H, BPC, N))
                    )
                    nc.tensor.matmul(
                        ps[:].rearrange("k (b n) -> k b n", b=BPC),
                        w2[p],
                        S_bc,
                        start=False,
                        stop=True,
                    )
                    ph.append(ps)
                tanh_and_sum(h_out, sb_S[p + 1], ph, sb_b[:, 1 + p : 2 + p])

            # Head: token-major [100, 8, 6]; cols 0..4 = logits, 5 = value.
            psh = pssm.tile([N, BLOC, A + 1], F32, name="ps_head", tag="ps_head")
            for b in range(BLOC):
                nc.tensor.matmul(
                    psh[:, b, :],
                    sb_h2[:, b * N : (b + 1) * N],
                    w_hv,
                    start=True,
                    stop=True,
                )
            if has_hvb:
                sb_hvb = persist.tile([N, A + 1], F32, name="sb_hvb")
                nc.gpsimd.dma_start(
                    out=sb_hvb[:], in_=hvb[:, :].to_broadcast((N, A + 1))
                )
                sb_log = small.tile([N, BLOC, A + 1], F32, name="sb_log")
                nc.vector.tensor_tensor(
                    sb_log[:],
                    psh[:],
                    sb_hvb[:].unsqueeze(1).to_broadcast((N, BLOC, A + 1)),
                    op=ALU.add,
                )
                src = sb_log
            else:
                src = psh

            # log_softmax over the A=5 logits (bounded, no max-sub needed).
            sb_exp = small.tile([N, BLOC, A + 1], F32, name="sb_exp")
            nc.scalar.activation(out=sb_exp[:], in_=src[:], func=AF.Exp)
            sb_sum = small.tile([N, BLOC], F32, name="sb_sum")
            nc.vector.tensor_reduce(
                sb_sum[:], sb_exp[:, :, 0:A], axis=AX.X, op=ALU.add
            )

            # ln(s) on VectorE: s = 2^e * m, ln = e*ln2 + poly(m), m in [1,2).
            s_i = sb_sum[:].bitcast(I32)
            t_e = small.tile([N, BLOC], I32, name="t_e")
            nc.vector.tensor_scalar(
                out=t_e[:], in0=s_i, scalar1=23, scalar2=None,
                op0=ALU.logical_shift_right,
            )
            t_ef = small.tile([N, BLOC], F32, name="t_ef")
            nc.vector.tensor_copy(t_ef[:], t_e[:])          # int32 -> f32 cast
            t_m = small.tile([N, BLOC], I32, name="t_m")
            nc.vector.tensor_scalar(
                out=t_m[:], in0=s_i, scalar1=0x007FFFFF, scalar2=0x3F800000,
                op0=ALU.bitwise_and, op1=ALU.bitwise_or,
            )
            m = t_m[:].bitcast(F32)
            c = LN_COEF
            deg = len(c) - 1
            # y = (((0 + c_n)*m + c_{n-1})*m ... + c_1)*m ; ln = y + c0 + e*ln2
            t_y = small.tile([N, BLOC], F32, name="t_y")
            nc.vector.tensor_scalar(
                out=t_y[:], in0=m, scalar1=c[deg], scalar2=None, op0=ALU.mult
            )
            for k in range(deg - 1, 0, -1):
                nc.vector.scalar_tensor_tensor(
                    out=t_y[:], in0=t_y[:], scalar=c[k], in1=m,
                    op0=ALU.add, op1=ALU.mult,
                )
            # lnS = e'*ln2 + y + C with C = c0 - 127*ln2 (e' = raw exponent).
            # Fuse: t2 = e'*ln2 + y here; C folds into the subtract below.
            sb_ln = small.tile([N, BLOC], F32, name="sb_ln")
            nc.vector.scalar_tensor_tensor(
                out=sb_ln[:], in0=t_ef[:], scalar=LN2, in1=t_y[:],
                op0=ALU.mult, op1=ALU.add,
            )

            # Combined output tile: cols 0..4 log_softmax, col 5 value.
            # Value copy rides on ScalarE (idle after exp) off the DVE chain.
            sb_out = small.tile([N, BLOC, A + 1], F32, name="sb_out")
            nc.scalar.copy(sb_out[:, :, A : A + 1], src[:, :, A : A + 1])
            nc.vector.scalar_tensor_tensor(
                out=sb_out[:, :, 0:A],
                in0=src[:, :, 0:A],
                scalar=LN_COEF[0] - 127.0 * LN2,
                in1=sb_ln[:].unsqueeze(2).to_broadcast((N, BLOC, A)),
                op0=ALU.subtract,
                op1=ALU.subtract,
            )
            nc.sync.dma_start(out=outp[:, :, :], in_=sb_out[:])

    nc.compile()
    return nc


_NC_CACHE: dict = {}


def _get_nc(has_hvb: bool) -> bass.Bass:
    key = (has_hvb, MM_MODE)
    if key not in _NC_CACHE:
        _NC_CACHE[key] = _build(has_hvb)
    return _NC_CACHE[key]


def _numpy_reference(x, W_enc, b_enc, f_W, f_b, C_W, C_b, head_W, head_b,
                     val_W, val_b, comm_mask):
    x_enc = np.tanh(x @ W_enc + b_enc)
    h = x_enc
    for i in range(NPASS):
        comm = np.einsum("ji,bjm->bim", comm_mask, h)
        c = comm @ C_W[i] + C_b[i]
        h = np.tanh(x_enc + h @ f_W[i] + f_b[i] + c)
    value = h @ val_W + val_b
    logits = h @ head_W + head_b
    m = logits.max(-1, keepdims=True)
    lse = m + np.log(np.exp(logits - m).sum(-1, keepdims=True))
    return (logits - lse).astype(np.float32), value.astype(np.float32)


def _prepare(inputs):
    """Host-side prep: mask decomposition, weight folding, sharding."""
    f = lambda k: np.asarray(inputs[k], np.float32)
    x = f("x")
    W_enc, b_enc = f("W_enc"), f("b_enc")
    f_W, f_b = f("f_W"), f("f_b")
    C_W, C_b = f("C_W"), f("C_b")
    head_W, head_b = f("head_W"), f("head_b")
    val_W, val_b = f("val_W"), f("val_b")
    M = f("comm_mask")

    # Decompose comm_mask = a*ones + d*eye.
    a = float(M[0, 1]) if N > 1 else 0.0
    d = float(M[0, 0]) - a
    scale = max(1.0, float(np.abs(M).max()))
    affine = np.allclose(M, a * np.ones((N, N)) + d * np.eye(N),
                         atol=1e-6 * scale, rtol=0.0)
    if not affine:
        return None  # caller falls back to numpy

    ndt = _np_dt()
    cvt = lambda arr: np.ascontiguousarray(np.asarray(arr).astype(ndt))

    w1h = f_W + d * C_W                                   # [P, H, H]
    eye = np.eye(H, dtype=np.float32)
    # Layout: [wenc | I | w1_0+I | w2_0 | w1_1 | w2_1 | hvw]
    # (pass 0 has h == x_enc, so its identity skip folds into w1_0)
    wp = np.concatenate(
        [W_enc, eye,
         w1h[0] + eye, a * C_W[0], w1h[1], a * C_W[1],
         np.concatenate([head_W, val_W], axis=1)],
        axis=1,
    )                                                     # [H, WCOLS]
    bp = np.stack([b_enc, f_b[0] + C_b[0], f_b[1] + C_b[1]], axis=1)  # [H, 3]

    hvb = np.concatenate([head_b, val_b])[None, :].astype(np.float32)
    has_hvb = bool(np.any(hvb))

    shared = {
        "wpack": cvt(wp),
        "bpack": np.ascontiguousarray(bp, np.float32),
    }
    if has_hvb:
        shared["hvb"] = hvb

    in_maps = []
    for c in range(NCORES):
        xc = x[c * BLOC : (c + 1) * BLOC].reshape(T, DIN)
        in_maps.append({**shared, "xT": cvt(xc.T)})
    return has_hvb, in_maps


def _postprocess(results):
    full = np.concatenate(
        [r["outp"].transpose(1, 0, 2) for r in results], axis=0
    )  # [B, N, A+1]
    action = np.ascontiguousarray(full[:, :, 0:A], dtype=np.float32)
    value = np.ascontiguousarray(full[:, :, A : A + 1], dtype=np.float32)
    return action, value


def run_traced(inputs, **kwargs):
    """Like kernel() but returns (outputs, BassKernelResults) with trace."""
    prep = _prepare(inputs)
    assert prep is not None
    has_hvb, in_maps = prep
    nc = _get_nc(has_hvb)
    res = run_bass_kernel_spmd(
        nc, in_maps, core_ids=list(range(NCORES)), trace=True, **kwargs
    )
    return _postprocess(res.results), res


def kernel(**inputs):
    prep = _prepare(inputs)
    if prep is None:
        return _numpy_reference(
            **{k: np.asarray(v, np.float32) for k, v in inputs.items()}
        )
    has_hvb, in_maps = prep
    nc = _get_nc(has_hvb)
    res = run_bass_kernel_spmd(nc, in_maps, core_ids=list(range(NCORES)))
    return _postprocess(res.results)


# revision 55
# speedup vs baseline: 1.0115x; 1.0115x over previous
"""CommNetMLP forward pass on Trainium2 (Bass/Tile), 8-core data-parallel.

Model (per reference):
    x_enc = tanh(x @ W_enc + b_enc)                      [B, N, H]
    h = x_enc
    for i in range(P):
        comm = einsum('ji,bjm->bim', comm_mask, h)       sender-sum
        c = comm @ C_W[i] + C_b[i]
        h = tanh(x_enc + h @ f_W[i] + f_b[i] + c)
    value  = h @ val_W + val_b                           [B, N, 1]
    action = log_softmax(h @ head_W + head_b, -1)        [B, N, A]

Device strategy (per core, B_loc = 8 batches = 800 tokens):
  comm_mask is (in practice) a*ones + d*eye, so
      comm[b,i] = a*S[b] + d*h[b,i],  S[b] = sum_j h[b,j]
  and each pass folds to
      h' = tanh(x_enc + h @ (f_W + d*C_W) + (a*S @ C_W)|bcast + (f_b + C_b))
  Activations live H-major [128, 800] so every matmul contracts over
  partitions.  Per pass, per 400-token chunk (4 batches):
    - PSUM accumulates h @ W1, the x_enc skip (identity matmul; folded into
      W1 host-side for pass 0 where h == x_enc), and the comm term as ONE
      matmul with a step-0 broadcast rhs: psum += W2'.T stationary x S[:, b]
      streamed 100x per batch.  Chunk c only needs chunk c's S columns, so
      stages pipeline per-chunk with no cross-chunk barrier.
    - ScalarE applies tanh(psum + bias); VectorE reduces the per-batch
      sender sums S for the next stage.
  The head runs token-major [100, 8, 6]; log_softmax reduces along the free
  dim, with ln() computed on the VectorE via exponent/mantissa bit
  extraction + a Horner polynomial (avoids a second ACT table load).
  Matmuls use float32r (single-pass, ~TF32 precision; 4x faster than fp32).
  ScalarE preloads its activation table and dummy matmuls warm the PE's HAM
  clock gate during the input DMAs.
"""

import numpy as np
import ml_dtypes

import concourse.bass as bass
from concourse import bacc
import concourse.mybir as mybir
import concourse.tile as tile
from concourse.bass_utils import run_bass_kernel_spmd

# Problem shapes (fixed for this problem instance).
B, N, H, DIN, A, NPASS = 64, 100, 128, 128, 5, 2
NCORES = 8
BLOC = B // NCORES            # batches per core
T = BLOC * N                  # tokens per core
NCHUNK = 2
CHUNK = T // NCHUNK           # matmul free-dim chunk (<= 512, batch aligned)
BPC = CHUNK // N              # batches per chunk

WCOLS = 6 * H + (A + 1)       # packed weights: wenc|I|w1_0+I|w2_0|w1_1|w2_1|hvw

F32 = mybir.dt.float32
BF16 = mybir.dt.bfloat16
I32 = mybir.dt.int32

# Matmul / activation storage dtype ("f32", "bf16", or "f32r").
MM_MODE = "f32r"

AF = mybir.ActivationFunctionType
ALU = mybir.AluOpType
AX = mybir.AxisListType

LN2 = float(np.log(2.0))

def _ln_poly_coeffs(deg=4):
    # Chebyshev fit of ln(x) on [1, 2], converted to power basis.
    cheb = np.polynomial.chebyshev.Chebyshev.fit(
        np.linspace(1.0, 2.0, 2048), np.log(np.linspace(1.0, 2.0, 2048)), deg
    )
    return [float(c) for c in cheb.convert(kind=np.polynomial.Polynomial).coef]

LN_COEF = _ln_poly_coeffs()   # c0..c7


def _mm_dt():
    return {"f32": F32, "bf16": BF16, "f32r": mybir.dt.float32r}[MM_MODE]


def _np_dt():
    return {"f32": np.float32, "bf16": ml_dtypes.bfloat16, "f32r": np.float32}[
        MM_MODE
    ]


def _build(has_hvb: bool) -> bass.Bass:
    DT = _mm_dt()
    nc = bacc.Bacc("TRN2", target_bir_lowering=False, num_swdge_queues=2)

    xT = nc.dram_tensor("xT", [H, T], DT, kind="ExternalInput")
    wpack = nc.dram_tensor("wpack", [H, WCOLS], DT, kind="ExternalInput")
    bpack = nc.dram_tensor("bpack", [H, 1 + NPASS], F32, kind="ExternalInput")
    if has_hvb:
        hvb = nc.dram_tensor("hvb", [1, A + 1], F32, kind="ExternalInput")

    outp = nc.dram_tensor("outp", [N, BLOC, A + 1], F32, kind="ExternalOutput")

    with tile.TileContext(nc) as tc:
        with (
            tc.tile_pool(name="persist", bufs=1) as persist,
            tc.tile_pool(name="small", bufs=2) as small,
            tc.tile_pool(name="psch", bufs=4, space="PSUM") as psch,
            tc.tile_pool(name="pssm", bufs=1, space="PSUM") as pssm,
        ):
            sb_x = persist.tile([H, T], DT, name="sb_x")
            sb_xenc = persist.tile([H, T], DT, name="sb_xenc")
            sb_h1 = persist.tile([H, T], DT, name="sb_h1")
            sb_h2 = persist.tile([H, T], DT, name="sb_h2")
            sb_w = persist.tile([H, WCOLS], DT, name="sb_w")
            sb_b = persist.tile([H, 1 + NPASS], F32, name="sb_b")
            # Per-stage per-batch sender sums S[k, b] (DT, per-chunk reduces).
            sb_S = [
                persist.tile([H, BLOC], DT, name=f"sb_S{s}") for s in range(3)
            ]

            # wpack column layout (ordered by when each section is needed):
            # [wenc | I | w1_0 | w2_0 | w1_1 | w2_1 | hvw]
            w_enc = sb_w[:, 0:H]
            w_id = sb_w[:, H : 2 * H]
            w1 = [sb_w[:, 2 * H : 3 * H], sb_w[:, 4 * H : 5 * H]]
            w2 = [sb_w[:, 3 * H : 4 * H], sb_w[:, 5 * H : 6 * H]]
            w_hv = sb_w[:, 6 * H : 6 * H + A + 1]
            b_enc = sb_b[:, 0:1]

            # Warmup: force the exp_and_others ACT table load to start at t=0.
            scratch = persist.tile([H, 1], F32, name="scratch")
            nc.vector.memset(scratch[:], 0.0)
            warm = persist.tile([H, 1], F32, name="warm")
            nc.scalar.activation(out=warm[:], in_=scratch[:], func=AF.Tanh)

            # PE warm-up: keep the systolic array busy from t~0 so the HAM
            # clock gate releases (1.2 -> 2.4 GHz) before the real matmuls.
            zeros32 = persist.tile([H, 512], F32, name="zeros32")
            nc.vector.memset(zeros32[:], 0.0)
            zeros = persist.tile([H, 512], DT, name="zeros")
            nc.vector.tensor_copy(zeros[:], zeros32[:])
            ps_wm = pssm.tile([1, 512], F32, name="ps_wm", tag="ps_wm")
            for _ in range(4):
                nc.tensor.matmul(
                    ps_wm[:], zeros[:, 0:1], zeros[:], start=True, stop=True
                )

            # Input DMAs spread across per-engine DGE queues (each HWDGE DMA
            # pays ~625ns fixed; transfers share one DMA bus, so order
            # follows need: x_c0, wenc+I, x_c1 on HWDGE; small/late
            # sections on the separate SWDGE (gpsimd) path).  This layout
            # measured best across all orderings tried.
            nc.sync.dma_start(out=sb_x[:, 0:CHUNK], in_=xT[:, 0:CHUNK])
            nc.sync.dma_start(out=sb_x[:, CHUNK:T], in_=xT[:, CHUNK:T])
            nc.scalar.dma_start(out=sb_w[:, 0 : 2 * H], in_=wpack[:, 0 : 2 * H])
            nc.gpsimd.dma_start(out=sb_b[:], in_=bpack[:, :])
            nc.gpsimd.dma_start(
                out=sb_w[:, 2 * H : 4 * H], in_=wpack[:, 2 * H : 4 * H]
            )
            nc.gpsimd.dma_start(
                out=sb_w[:, 4 * H : WCOLS], in_=wpack[:, 4 * H : WCOLS]
            )

            def tanh_and_sum(h_out, S_out, psums, bias):
                """Per-chunk tanh (ScalarE) + per-chunk per-batch sender-sum
                reduce (VectorE) writing S columns for the next stage."""
                for c in range(NCHUNK):
                    sl = slice(c * CHUNK, (c + 1) * CHUNK)
                    nc.scalar.activation(
                        out=h_out[:, sl], in_=psums[c][:], func=AF.Tanh,
                        bias=bias,
                    )
                    with nc.allow_low_precision(reason="f32r sender-sum"):
                        nc.vector.tensor_reduce(
                            S_out[:, c * BPC : (c + 1) * BPC],
                            h_out[:, sl].rearrange("k (b n) -> k b n", b=BPC),
                            axis=AX.X,
                            op=ALU.add,
                        )

            # Encoder.
            enc_ps = []
            for c in range(NCHUNK):
                sl = slice(c * CHUNK, (c + 1) * CHUNK)
                ps = psch.tile([H, CHUNK], F32, name="ps_mm", tag="ps_mm")
                nc.tensor.matmul(ps[:], w_enc, sb_x[:, sl], start=True, stop=True)
                enc_ps.append(ps)
            tanh_and_sum(sb_xenc, sb_S[0], enc_ps, b_enc)

            # Communication passes.  The comm term lands in PSUM as a single
            # matmul per chunk with a step-0 broadcast rhs:
            #   psum[hout, (b, n)] += sum_k W2'[k, hout] * S[k, b]
            # and chunk c only needs chunk c's own S columns, so the pass-to-
            # pass joint is fully per-chunk pipelined.
            for p in range(NPASS):
                h_in = sb_xenc if p == 0 else sb_h1
                h_out = sb_h1 if p == 0 else sb_h2

                ph = []
                for c in range(NCHUNK):
                    sl = slice(c * CHUNK, (c + 1) * CHUNK)
                    ps = psch.tile([H, CHUNK], F32, name="ps_mm", tag="ps_mm")
                    if p == 0:
                        # h_in == x_enc, so the identity skip is folded into
                        # w1[0] host-side (W1_0 + I): one matmul instead of 2.
                        nc.tensor.matmul(
                            ps[:], w1[p], h_in[:, sl], start=True, stop=False
                        )
                    else:
                        nc.tensor.matmul(
                            ps[:], w_id, sb_xenc[:, sl], start=True, stop=False
                        )
                        nc.tensor.matmul(
                            ps[:], w1[p], h_in[:, sl], start=False, stop=False
                        )
                    S_bc = (
                        sb_S[p][:, c * BPC : (c + 1) * BPC]
                        .unsqueeze(2)
                        .to_broadcast((H, BPC, N))
                    )
                    nc.tensor.matmul(
                        ps[:].rearrange("k (b n) -> k b n", b=BPC),
                        w2[p],
                        S_bc,
                        start=False,
                        stop=True,
                    )
                    ph.append(ps)
                tanh_and_sum(h_out, sb_S[p + 1], ph, sb_b[:, 1 + p : 2 + p])

            # Head: token-major [100, 8, 6]; cols 0..4 = logits, 5 = value.
            psh = pssm.tile([N, BLOC, A + 1], F32, name="ps_head", tag="ps_head")
            for b in range(BLOC):
                nc.tensor.matmul(
                    psh[:, b, :],
                    sb_h2[:, b * N : (b + 1) * N],
                    w_hv,
                    start=True,
                    stop=True,
                )
            if has_hvb:
                sb_hvb = persist.tile([N, A + 1], F32, name="sb_hvb")
                nc.gpsimd.dma_start(
                    out=sb_hvb[:], in_=hvb[:, :].to_broadcast((N, A + 1))
                )
                sb_log = small.tile([N, BLOC, A + 1], F32, name="sb_log")
                nc.vector.tensor_tensor(
                    sb_log[:],
                    psh[:],
                    sb_hvb[:].unsqueeze(1).to_broadcast((N, BLOC, A + 1)),
                    op=ALU.add,
                )
                src = sb_log
            else:
                src = psh

            # log_softmax over the A=5 logits (bounded, no max-sub needed).
            sb_exp = small.tile([N, BLOC, A + 1], F32, name="sb_exp")
            nc.scalar.activation(out=sb_exp[:], in_=src[:], func=AF.Exp)
            sb_sum = small.tile([N, BLOC], F32, name="sb_sum")
            nc.vector.tensor_reduce(
                sb_sum[:], sb_exp[:, :, 0:A], axis=AX.X, op=ALU.add
            )

            # ln(s) on VectorE: s = 2^e * m, ln = e*ln2 + poly(m), m in [1,2).
            s_i = sb_sum[:].bitcast(I32)
            t_e = small.tile([N, BLOC], I32, name="t_e")
            nc.vector.tensor_scalar(
                out=t_e[:], in0=s_i, scalar1=23, scalar2=None,
                op0=ALU.logical_shift_right,
            )
            t_ef = small.tile([N, BLOC], F32, name="t_ef")
            nc.vector.tensor_copy(t_ef[:], t_e[:])          # int32 -> f32 cast
            t_m = small.tile([N, BLOC], I32, name="t_m")
            nc.vector.tensor_scalar(
                out=t_m[:], in0=s_i, scalar1=0x007FFFFF, scalar2=0x3F800000,
                op0=ALU.bitwise_and, op1=ALU.bitwise_or,
            )
            m = t_m[:].bitcast(F32)
            c = LN_COEF
            deg = len(c) - 1
            # y = (((0 + c_n)*m + c_{n-1})*m ... + c_1)*m ; ln = y + c0 + e*ln2
            t_y = small.tile([N, BLOC], F32, name="t_y")
            nc.vector.tensor_scalar(
                out=t_y[:], in0=m, scalar1=c[deg], scalar2=None, op0=ALU.mult
            )
            for k in range(deg - 1, 0, -1):
                nc.vector.scalar_tensor_tensor(
                    out=t_y[:], in0=t_y[:], scalar=c[k], in1=m,
                    op0=ALU.add, op1=ALU.mult,
                )
            # lnS = e'*ln2 + y + C with C = c0 - 127*ln2 (e' = raw exponent).
            # Fuse: t2 = e'*ln2 + y here; C folds into the subtract below.
            sb_ln = small.tile([N, BLOC], F32, name="sb_ln")
            nc.vector.scalar_tensor_tensor(
                out=sb_ln[:], in0=t_ef[:], scalar=LN2, in1=t_y[:],
                op0=ALU.mult, op1=ALU.add,
            )

            # Combined output tile: cols 0..4 log_softmax, col 5 value.
            # Value copy rides on ScalarE (idle after exp) off the DVE chain.
            sb_out = small.tile([N, BLOC, A + 1], F32, name="sb_out")
            nc.scalar.copy(sb_out[:, :, A : A + 1], src[:, :, A : A + 1])
            nc.vector.scalar_tensor_tensor(
                out=sb_out[:, :, 0:A],
                in0=src[:, :, 0:A],
                scalar=LN_COEF[0] - 127.0 * LN2,
                in1=sb_ln[:].unsqueeze(2).to_broadcast((N, BLOC, A)),
                op0=ALU.subtract,
                op1=ALU.subtract,
            )
            nc.sync.dma_start(out=outp[:, :, :], in_=sb_out[:])

    nc.compile()
    return nc


_NC_CACHE: dict = {}


def _get_nc(has_hvb: bool) -> bass.Bass:
    key = (has_hvb, MM_MODE)
    if key not in _NC_CACHE:
        _NC_CACHE[key] = _build(has_hvb)
    return _NC_CACHE[key]


def _numpy_reference(x, W_enc, b_enc, f_W, f_b, C_W, C_b, head_W, head_b,
                     val_W, val_b, comm_mask):
    x_enc = np.tanh(x @ W_enc + b_enc)
    h = x_enc
    for i in range(NPASS):
        comm = np.einsum("ji,bjm->bim", comm_mask, h)
        c = comm @ C_W[i] + C_b[i]
        h = np.tanh(x_enc + h @ f_W[i] + f_b[i] + c)
    value = h @ val_W + val_b
    logits = h @ head_W + head_b
    m = logits.max(-1, keepdims=True)
    lse = m + np.log(np.exp(logits - m).sum(-1, keepdims=True))
    return (logits - lse).astype(np.float32), value.astype(np.float32)


def _prepare(inputs):
    """Host-side prep: mask decomposition, weight folding, sharding."""
    f = lambda k: np.asarray(inputs[k], np.float32)
    x = f("x")
    W_enc, b_enc = f("W_enc"), f("b_enc")
    f_W, f_b = f("f_W"), f("f_b")
    C_W, C_b = f("C_W"), f("C_b")
    head_W, head_b = f("head_W"), f("head_b")
    val_W, val_b = f("val_W"), f("val_b")
    M = f("comm_mask")

    # Decompose comm_mask = a*ones + d*eye.
    a = float(M[0, 1]) if N > 1 else 0.0
    d = float(M[0, 0]) - a
    scale = max(1.0, float(np.abs(M).max()))
    affine = np.allclose(M, a * np.ones((N, N)) + d * np.eye(N),
                         atol=1e-6 * scale, rtol=0.0)
    if not affine:
        return None  # caller falls back to numpy

    ndt = _np_dt()
    cvt = lambda arr: np.ascontiguousarray(np.asarray(arr).astype(ndt))

    w1h = f_W + d * C_W                                   # [P, H, H]
    eye = np.eye(H, dtype=np.float32)
    # Layout: [wenc | I | w1_0+I | w2_0 | w1_1 | w2_1 | hvw]
    # (pass 0 has h == x_enc, so its identity skip folds into w1_0)
    wp = np.concatenate(
        [W_enc, eye,
         w1h[0] + eye, a * C_W[0], w1h[1], a * C_W[1],
         np.concatenate([head_W, val_W], axis=1)],
        axis=1,
    )                                                     # [H, WCOLS]
    bp = np.stack([b_enc, f_b[0] + C_b[0], f_b[1] + C_b[1]], axis=1)  # [H, 3]

    hvb = np.concatenate([head_b, val_b])[None, :].astype(np.float32)
    has_hvb = bool(np.any(hvb))

    shared = {
        "wpack": cvt(wp),
        "bpack": np.ascontiguousarray(bp, np.float32),
    }
    if has_hvb:
        shared["hvb"] = hvb

    in_maps = []
    for c in range(NCORES):
        xc = x[c * BLOC : (c + 1) * BLOC].reshape(T, DIN)
        in_maps.append({**shared, "xT": cvt(xc.T)})
    return has_hvb, in_maps


def _postprocess(results):
    full = np.concatenate(
        [r["outp"].transpose(1, 0, 2) for r in results], axis=0
    )  # [B, N, A+1]
    action = np.ascontiguousarray(full[:, :, 0:A], dtype=np.float32)
    value = np.ascontiguousarray(full[:, :, A : A + 1], dtype=np.float32)
    return action, value


def run_traced(inputs, **kwargs):
    """Like kernel() but returns (outputs, BassKernelResults) with trace."""
    prep = _prepare(inputs)
    assert prep is not None
    has_hvb, in_maps = prep
    nc = _get_nc(has_hvb)
    res = run_bass_kernel_spmd(
        nc, in_maps, core_ids=list(range(NCORES)), trace=True, **kwargs
    )
    return _postprocess(res.results), res


def kernel(**inputs):
    prep = _prepare(inputs)
    if prep is None:
        return _numpy_reference(
            **{k: np.asarray(v, np.float32) for k, v in inputs.items()}
        )
    has_hvb, in_maps = prep
    nc = _get_nc(has_hvb)
    res = run_bass_kernel_spmd(nc, in_maps, core_ids=list(range(NCORES)))
    return _postprocess(res.results)


# revision 59
# speedup vs baseline: 1.0398x; 1.0279x over previous
"""CommNetMLP forward pass on Trainium2 (Bass/Tile), 8-core data-parallel.

Model (per reference):
    x_enc = tanh(x @ W_enc + b_enc)                      [B, N, H]
    h = x_enc
    for i in range(P):
        comm = einsum('ji,bjm->bim', comm_mask, h)       sender-sum
        c = comm @ C_W[i] + C_b[i]
        h = tanh(x_enc + h @ f_W[i] + f_b[i] + c)
    value  = h @ val_W + val_b                           [B, N, 1]
    action = log_softmax(h @ head_W + head_b, -1)        [B, N, A]

Device strategy (per core, B_loc = 8 batches = 800 tokens):
  comm_mask is (in practice) a*ones + d*eye, so
      comm[b,i] = a*S[b] + d*h[b,i],  S[b] = sum_j h[b,j]
  and each pass folds to
      h' = tanh(x_enc + h @ (f_W + d*C_W) + (a*S @ C_W)|bcast + (f_b + C_b))
  Activations live H-major [128, 800] so every matmul contracts over
  partitions.  Per pass, per 400-token chunk (4 batches):
    - PSUM accumulates h @ W1, the x_enc skip (identity matmul; folded into
      W1 host-side for pass 0 where h == x_enc), and the comm term as ONE
      matmul with a step-0 broadcast rhs: psum += W2'.T stationary x S[:, b]
      streamed 100x per batch.  Chunk c only needs chunk c's S columns, so
      stages pipeline per-chunk with no cross-chunk barrier.
    - ScalarE applies tanh(psum + bias); VectorE reduces the per-batch
      sender sums S for the next stage.
  The head runs token-major [100, 8, 6]; log_softmax reduces along the free
  dim, with ln() computed on the VectorE via exponent/mantissa bit
  extraction + a Horner polynomial (avoids a second ACT table load).
  Matmuls use float32r (single-pass, ~TF32 precision; 4x faster than fp32).
  ScalarE preloads its activation table and dummy matmuls warm the PE's HAM
  clock gate during the input DMAs.
"""

import numpy as np
import ml_dtypes

import concourse.bass as bass
from concourse import bacc
import concourse.mybir as mybir
import concourse.tile as tile
from concourse.bass_utils import run_bass_kernel_spmd

# Problem shapes (fixed for this problem instance).
B, N, H, DIN, A, NPASS = 64, 100, 128, 128, 5, 2
NCORES = 8
BLOC = B // NCORES            # batches per core
T = BLOC * N                  # tokens per core
NCHUNK = 2
CHUNK = T // NCHUNK           # matmul free-dim chunk (<= 512, batch aligned)
BPC = CHUNK // N              # batches per chunk

WCOLS = 6 * H + (A + 1)       # packed weights: wenc|I|w1_0+I|w2_0|w1_1|w2_1|hvw

F32 = mybir.dt.float32
BF16 = mybir.dt.bfloat16
I32 = mybir.dt.int32

# Matmul / activation storage dtype ("f32", "bf16", or "f32r").
MM_MODE = "f32r"

AF = mybir.ActivationFunctionType
ALU = mybir.AluOpType
AX = mybir.AxisListType

LN2 = float(np.log(2.0))

def _ln_poly_coeffs(deg=4):
    # Chebyshev fit of ln(x) on [1, 2], converted to power basis.
    cheb = np.polynomial.chebyshev.Chebyshev.fit(
        np.linspace(1.0, 2.0, 2048), np.log(np.linspace(1.0, 2.0, 2048)), deg
    )
    return [float(c) for c in cheb.convert(kind=np.polynomial.Polynomial).coef]

LN_COEF = _ln_poly_coeffs()   # c0..c7


def _mm_dt():
    return {"f32": F32, "bf16": BF16, "f32r": mybir.dt.float32r}[MM_MODE]


def _np_dt():
    return {"f32": np.float32, "bf16": ml_dtypes.bfloat16, "f32r": np.float32}[
        MM_MODE
    ]


def _build(has_hvb: bool) -> bass.Bass:
    DT = _mm_dt()
    nc = bacc.Bacc("TRN2", target_bir_lowering=False, num_swdge_queues=2)

    xT = nc.dram_tensor("xT", [H, T], DT, kind="ExternalInput")
    wpack = nc.dram_tensor("wpack", [H, WCOLS], DT, kind="ExternalInput")
    bpack = nc.dram_tensor("bpack", [H, 1 + NPASS], F32, kind="ExternalInput")
    if has_hvb:
        hvb = nc.dram_tensor("hvb", [1, A + 1], F32, kind="ExternalInput")

    outp = nc.dram_tensor("outp", [N, BLOC, A + 1], F32, kind="ExternalOutput")

    with tile.TileContext(nc) as tc:
        with (
            tc.tile_pool(name="persist", bufs=1) as persist,
            tc.tile_pool(name="small", bufs=2) as small,
            tc.tile_pool(name="psch", bufs=4, space="PSUM") as psch,
            tc.tile_pool(name="pssm", bufs=1, space="PSUM") as pssm,
        ):
            sb_x = persist.tile([H, T], DT, name="sb_x")
            sb_xenc = persist.tile([H, T], DT, name="sb_xenc")
            sb_h1 = persist.tile([H, T], DT, name="sb_h1")
            sb_h2 = persist.tile([H, T], DT, name="sb_h2")
            sb_w = persist.tile([H, WCOLS], DT, name="sb_w")
            sb_b = persist.tile([H, 1 + NPASS], F32, name="sb_b")
            # Per-stage per-batch sender sums S[k, b] (DT, per-chunk reduces).
            sb_S = [
                persist.tile([H, BLOC], DT, name=f"sb_S{s}")
                for s in range(NPASS)
            ]

            # wpack column layout (ordered by when each section is needed):
            # [wenc | I | w1_0 | w2_0 | w1_1 | w2_1 | hvw]
            w_enc = sb_w[:, 0:H]
            w_id = sb_w[:, H : 2 * H]
            w1 = [sb_w[:, 2 * H : 3 * H], sb_w[:, 4 * H : 5 * H]]
            w2 = [sb_w[:, 3 * H : 4 * H], sb_w[:, 5 * H : 6 * H]]
            w_hv = sb_w[:, 6 * H : 6 * H + A + 1]
            b_enc = sb_b[:, 0:1]

            # Warmup: force the exp_and_others ACT table load to start at t=0.
            scratch = persist.tile([H, 1], F32, name="scratch")
            nc.vector.memset(scratch[:], 0.0)
            warm = persist.tile([H, 1], F32, name="warm")
            nc.scalar.activation(out=warm[:], in_=scratch[:], func=AF.Tanh)

            # PE warm-up: keep the systolic array busy from t~0 so the HAM
            # clock gate releases (1.2 -> 2.4 GHz) before the real matmuls.
            zeros32 = persist.tile([H, 512], F32, name="zeros32")
            nc.vector.memset(zeros32[:], 0.0)
            zeros = persist.tile([H, 512], DT, name="zeros")
            nc.vector.tensor_copy(zeros[:], zeros32[:])
            ps_wm = pssm.tile([1, 512], F32, name="ps_wm", tag="ps_wm")
            for _ in range(2):
                nc.tensor.matmul(
                    ps_wm[:], zeros[:, 0:1], zeros[:], start=True, stop=True
                )

            # Input DMAs spread across per-engine DGE queues (each HWDGE DMA
            # pays ~625ns fixed; transfers share one DMA bus, so order
            # follows need: x_c0, wenc+I, x_c1 on HWDGE; small/late
            # sections on the separate SWDGE (gpsimd) path).  This layout
            # measured best across all orderings tried.
            nc.sync.dma_start(out=sb_x[:, 0:CHUNK], in_=xT[:, 0:CHUNK])
            nc.sync.dma_start(out=sb_x[:, CHUNK:T], in_=xT[:, CHUNK:T])
            nc.scalar.dma_start(out=sb_w[:, 0 : 2 * H], in_=wpack[:, 0 : 2 * H])
            nc.gpsimd.dma_start(out=sb_b[:], in_=bpack[:, :])
            nc.gpsimd.dma_start(
                out=sb_w[:, 2 * H : 4 * H], in_=wpack[:, 2 * H : 4 * H]
            )
            nc.gpsimd.dma_start(
                out=sb_w[:, 4 * H : WCOLS], in_=wpack[:, 4 * H : WCOLS]
            )

            def tanh_and_sum(h_out, S_out, psums, bias):
                """Per-chunk tanh (ScalarE) + per-chunk per-batch sender-sum
                reduce (VectorE) writing S columns for the next stage.
                S_out=None (last stage) skips the unused reduce."""
                for c in range(NCHUNK):
                    sl = slice(c * CHUNK, (c + 1) * CHUNK)
                    nc.scalar.activation(
                        out=h_out[:, sl], in_=psums[c][:], func=AF.Tanh,
                        bias=bias,
                    )
                    if S_out is None:
                        continue
                    with nc.allow_low_precision(reason="f32r sender-sum"):
                        nc.vector.tensor_reduce(
                            S_out[:, c * BPC : (c + 1) * BPC],
                            h_out[:, sl].rearrange("k (b n) -> k b n", b=BPC),
                            axis=AX.X,
                            op=ALU.add,
                        )

            # Encoder.
            enc_ps = []
            for c in range(NCHUNK):
                sl = slice(c * CHUNK, (c + 1) * CHUNK)
                ps = psch.tile([H, CHUNK], F32, name="ps_mm", tag="ps_mm")
                nc.tensor.matmul(ps[:], w_enc, sb_x[:, sl], start=True, stop=True)
                enc_ps.append(ps)
            tanh_and_sum(sb_xenc, sb_S[0], enc_ps, b_enc)

            # Communication passes.  The comm term lands in PSUM as a single
            # matmul per chunk with a step-0 broadcast rhs:
            #   psum[hout, (b, n)] += sum_k W2'[k, hout] * S[k, b]
            # and chunk c only needs chunk c's own S columns, so the pass-to-
            # pass joint is fully per-chunk pipelined.
            for p in range(NPASS):
                h_in = sb_xenc if p == 0 else sb_h1
                h_out = sb_h1 if p == 0 else sb_h2

                ph = []
                for c in range(NCHUNK):
                    sl = slice(c * CHUNK, (c + 1) * CHUNK)
                    ps = psch.tile([H, CHUNK], F32, name="ps_mm", tag="ps_mm")
                    if p == 0:
                        # h_in == x_enc, so the identity skip is folded into
                        # w1[0] host-side (W1_0 + I): one matmul instead of 2.
                        nc.tensor.matmul(
                            ps[:], w1[p], h_in[:, sl], start=True, stop=False
                        )
                    else:
                        nc.tensor.matmul(
                            ps[:], w_id, sb_xenc[:, sl], start=True, stop=False
                        )
                        nc.tensor.matmul(
                            ps[:], w1[p], h_in[:, sl], start=False, stop=False
                        )
                    S_bc = (
                        sb_S[p][:, c * BPC : (c + 1) * BPC]
                        .unsqueeze(2)
                        .to_broadcast((H, BPC, N))
                    )
                    nc.tensor.matmul(
                        ps[:].rearrange("k (b n) -> k b n", b=BPC),
                        w2[p],
                        S_bc,
                        start=False,
                        stop=True,
                    )
                    ph.append(ps)
                S_next = sb_S[p + 1] if p + 1 < NPASS else None
                tanh_and_sum(h_out, S_next, ph, sb_b[:, 1 + p : 2 + p])

            # Head: token-major [100, 8, 6]; cols 0..4 = logits, 5 = value.
            psh = pssm.tile([N, BLOC, A + 1], F32, name="ps_head", tag="ps_head")
            for b in range(BLOC):
                nc.tensor.matmul(
                    psh[:, b, :],
                    sb_h2[:, b * N : (b + 1) * N],
                    w_hv,
                    start=True,
                    stop=True,
                )
            if has_hvb:
                sb_hvb = persist.tile([N, A + 1], F32, name="sb_hvb")
                nc.gpsimd.dma_start(
                    out=sb_hvb[:], in_=hvb[:, :].to_broadcast((N, A + 1))
                )
                sb_log = small.tile([N, BLOC, A + 1], F32, name="sb_log")
                nc.vector.tensor_tensor(
                    sb_log[:],
                    psh[:],
                    sb_hvb[:].unsqueeze(1).to_broadcast((N, BLOC, A + 1)),
                    op=ALU.add,
                )
                src = sb_log
            else:
                src = psh

            # log_softmax over the A=5 logits (bounded, no max-sub needed).
            sb_exp = small.tile([N, BLOC, A + 1], F32, name="sb_exp")
            nc.scalar.activation(out=sb_exp[:], in_=src[:], func=AF.Exp)
            sb_sum = small.tile([N, BLOC], F32, name="sb_sum")
            nc.vector.tensor_reduce(
                sb_sum[:], sb_exp[:, :, 0:A], axis=AX.X, op=ALU.add
            )

            # ln(s) on VectorE: s = 2^e * m, ln = e*ln2 + poly(m), m in [1,2).
            s_i = sb_sum[:].bitcast(I32)
            t_e = small.tile([N, BLOC], I32, name="t_e")
            nc.vector.tensor_scalar(
                out=t_e[:], in0=s_i, scalar1=23, scalar2=None,
                op0=ALU.logical_shift_right,
            )
            # Fused int->f32 cast + *ln2 (DVE ALU converts the int operand,
            # f32 out dtype converts on write).
            t_ef = small.tile([N, BLOC], F32, name="t_ef")
            nc.vector.tensor_scalar(
                out=t_ef[:], in0=t_e[:], scalar1=LN2, scalar2=None,
                op0=ALU.mult,
            )
            t_m = small.tile([N, BLOC], I32, name="t_m")
            nc.vector.tensor_scalar(
                out=t_m[:], in0=s_i, scalar1=0x007FFFFF, scalar2=0x3F800000,
                op0=ALU.bitwise_and, op1=ALU.bitwise_or,
            )
            m = t_m[:].bitcast(F32)
            c = LN_COEF
            deg = len(c) - 1
            # Normalized Horner: z = (m + c3/c4)*m, z = (z + c2/c4)*m, ...,
            # then lnS-partial = c4*z + e'*ln2; C = c0 - 127*ln2 folds into
            # the final subtract.
            t_y = small.tile([N, BLOC], F32, name="t_y")
            nc.vector.scalar_tensor_tensor(
                out=t_y[:], in0=m, scalar=c[deg - 1] / c[deg], in1=m,
                op0=ALU.add, op1=ALU.mult,
            )
            for k in range(deg - 2, 0, -1):
                nc.vector.scalar_tensor_tensor(
                    out=t_y[:], in0=t_y[:], scalar=c[k] / c[deg], in1=m,
                    op0=ALU.add, op1=ALU.mult,
                )
            sb_ln = small.tile([N, BLOC], F32, name="sb_ln")
            nc.vector.scalar_tensor_tensor(
                out=sb_ln[:], in0=t_y[:], scalar=c[deg], in1=t_ef[:],
                op0=ALU.mult, op1=ALU.add,
            )

            # Combined output tile: cols 0..4 log_softmax, col 5 value.
            # Value copy rides on ScalarE (idle after exp) off the DVE chain.
            sb_out = small.tile([N, BLOC, A + 1], F32, name="sb_out")
            nc.scalar.copy(sb_out[:, :, A : A + 1], src[:, :, A : A + 1])
            nc.vector.scalar_tensor_tensor(
                out=sb_out[:, :, 0:A],
                in0=src[:, :, 0:A],
                scalar=LN_COEF[0] - 127.0 * LN2,
                in1=sb_ln[:].unsqueeze(2).to_broadcast((N, BLOC, A)),
                op0=ALU.subtract,
                op1=ALU.subtract,
            )
            nc.sync.dma_start(out=outp[:, :, :], in_=sb_out[:])

    nc.compile()
    return nc


_NC_CACHE: dict = {}


def _get_nc(has_hvb: bool) -> bass.Bass:
    key = (has_hvb, MM_MODE)
    if key not in _NC_CACHE:
        _NC_CACHE[key] = _build(has_hvb)
    return _NC_CACHE[key]


def _numpy_reference(x, W_enc, b_enc, f_W, f_b, C_W, C_b, head_W, head_b,
                     val_W, val_b, comm_mask):
    x_enc = np.tanh(x @ W_enc + b_enc)
    h = x_enc
    for i in range(NPASS):
        comm = np.einsum("ji,bjm->bim", comm_mask, h)
        c = comm @ C_W[i] + C_b[i]
        h = np.tanh(x_enc + h @ f_W[i] + f_b[i] + c)
    value = h @ val_W + val_b
    logits = h @ head_W + head_b
    m = logits.max(-1, keepdims=True)
    lse = m + np.log(np.exp(logits - m).sum(-1, keepdims=True))
    return (logits - lse).astype(np.float32), value.astype(np.float32)


def _prepare(inputs):
    """Host-side prep: mask decomposition, weight folding, sharding."""
    f = lambda k: np.asarray(inputs[k], np.float32)
    x = f("x")
    W_enc, b_enc = f("W_enc"), f("b_enc")
    f_W, f_b = f("f_W"), f("f_b")
    C_W, C_b = f("C_W"), f("C_b")
    head_W, head_b = f("head_W"), f("head_b")
    val_W, val_b = f("val_W"), f("val_b")
    M = f("comm_mask")

    # Decompose comm_mask = a*ones + d*eye.
    a = float(M[0, 1]) if N > 1 else 0.0
    d = float(M[0, 0]) - a
    scale = max(1.0, float(np.abs(M).max()))
    affine = np.allclose(M, a * np.ones((N, N)) + d * np.eye(N),
                         atol=1e-6 * scale, rtol=0.0)
    if not affine:
        return None  # caller falls back to numpy

    ndt = _np_dt()
    cvt = lambda arr: np.ascontiguousarray(np.asarray(arr).astype(ndt))

    w1h = f_W + d * C_W                                   # [P, H, H]
    eye = np.eye(H, dtype=np.float32)
    # Layout: [wenc | I | w1_0+I | w2_0 | w1_1 | w2_1 | hvw]
    # (pass 0 has h == x_enc, so its identity skip folds into w1_0)
    wp = np.concatenate(
        [W_enc, eye,
         w1h[0] + eye, a * C_W[0], w1h[1], a * C_W[1],
         np.concatenate([head_W, val_W], axis=1)],
        axis=1,
    )                                                     # [H, WCOLS]
    bp = np.stack([b_enc, f_b[0] + C_b[0], f_b[1] + C_b[1]], axis=1)  # [H, 3]

    hvb = np.concatenate([head_b, val_b])[None, :].astype(np.float32)
    has_hvb = bool(np.any(hvb))

    shared = {
        "wpack": cvt(wp),
        "bpack": np.ascontiguousarray(bp, np.float32),
    }
    if has_hvb:
        shared["hvb"] = hvb

    in_maps = []
    for c in range(NCORES):
        xc = x[c * BLOC : (c + 1) * BLOC].reshape(T, DIN)
        in_maps.append({**shared, "xT": cvt(xc.T)})
    return has_hvb, in_maps


def _postprocess(results):
    full = np.concatenate(
        [r["outp"].transpose(1, 0, 2) for r in results], axis=0
    )  # [B, N, A+1]
    action = np.ascontiguousarray(full[:, :, 0:A], dtype=np.float32)
    value = np.ascontiguousarray(full[:, :, A : A + 1], dtype=np.float32)
    return action, value


def run_traced(inputs, **kwargs):
    """Like kernel() but returns (outputs, BassKernelResults) with trace."""
    prep = _prepare(inputs)
    assert prep is not None
    has_hvb, in_maps = prep
    nc = _get_nc(has_hvb)
    res = run_bass_kernel_spmd(
        nc, in_maps, core_ids=list(range(NCORES)), trace=True, **kwargs
    )
    return _postprocess(res.results), res


def kernel(**inputs):
    prep = _prepare(inputs)
    if prep is None:
        return _numpy_reference(
            **{k: np.asarray(v, np.float32) for k, v in inputs.items()}
        )
    has_hvb, in_maps = prep
    nc = _get_nc(has_hvb)
    res = run_bass_kernel_spmd(nc, in_maps, core_ids=list(range(NCORES)))
    return _postprocess(res.results)
